# revision 1
# baseline (speedup 1.0000x reference)
"""MoE GPT-OSS MLP block (expert-parallel) for 8 Trainium2 NeuronCores.

Strategy:
- Host: router (logits -> top-4 -> softmax), per-expert token gather with
  capacity padding, input transpose, weight padding to 2944 (=23*128).
- Device (SPMD, core e = expert e): yT = SwiGLU-MLP of xT in bf16 matmuls
  with fp32 PSUM accumulation; two token chunks keep x/h SBUF-resident so
  each weight matrix streams from HBM exactly once per chunk.
- Host: out[idx_e] += aff_e * yT.T  (affinity combine; unselected experts
  contribute exactly 0 in the reference, so skipping them is exact).
"""
import sys

sys.path.insert(0, "/opt/trn_rl_repo")

import numpy as np

TOP_K = 4
E = 8
H = 2880
D = 2944  # 23 * 128, zero-padded contraction/output dims
NT = D // 128  # 23 tiles
N_CORES = 8
MIN_CAP = 2176

_kernel_cache = {}
last_results = None  # stashed for test harness introspection


def _n_tiles(ch):
    """Split a chunk width into matmul N-tiles (<=512 each)."""
    out = []
    o = 0
    while o < ch:
        w = min(512, ch - o)
        out.append((o, w))
        o += w
    return out


def _build(C, chunks):
    import concourse.bacc as bacc
    import concourse.tile as tile
    from concourse import mybir

    f32 = mybir.dt.float32
    bf16 = mybir.dt.bfloat16

    nc = bacc.Bacc(None)
    xT = nc.declare_dram_parameter("xT", [D, C], f32, isOutput=False)
    wg = nc.declare_dram_parameter("wg", [D, D], f32, isOutput=False)
    wu = nc.declare_dram_parameter("wu", [D, D], f32, isOutput=False)
    wd = nc.declare_dram_parameter("wd", [D, D], f32, isOutput=False)
    yT = nc.declare_dram_parameter("yT", [D, C], f32, isOutput=True)

    # [part, ktile, col] views of the padded DRAM tensors
    xT_r = xT.rearrange("(kt p) t -> p kt t", p=128)
    wg_r = wg.rearrange("(kt p) i -> p kt i", p=128)
    wu_r = wu.rearrange("(kt p) i -> p kt i", p=128)
    wd_r = wd.rearrange("(kt p) h -> p kt h", p=128)

    with tile.TileContext(nc) as tc:
        with (
            tc.tile_pool(name="xp", bufs=2) as xp,
            tc.tile_pool(name="hp", bufs=1) as hp,
            tc.tile_pool(name="wp", bufs=2) as wp,
            tc.tile_pool(name="sp", bufs=2) as sp,
            tc.tile_pool(name="pp", bufs=1, space="PSUM") as pp,
        ):
            t0 = 0
            for ci, CH in enumerate(chunks):
                nsl = _n_tiles(CH)
                # ---- load x chunk (DMA-cast f32 -> bf16), one big DMA ----
                xc = xp.tile([128, NT, CH], bf16, name=f"xc{ci}", tag="xc")
                nc.gpsimd.dma_start(xc[:], xT_r[:, :, t0 : t0 + CH])

                hc = hp.tile([128, NT, CH], bf16, name=f"hc{ci}", tag="hc")

                # ---- phase A: g = x@wg, u = x@wu, h = silu(g)*u ----
                for im in range(NT):
                    sl = slice(im * 128, (im + 1) * 128)
                    wgt = wp.tile([128, NT, 128], bf16, name=f"wg_{ci}_{im}", tag="wgt")
                    nc.gpsimd.dma_start(wgt[:], wg_r[:, :, sl])
                    wut = wp.tile([128, NT, 128], bf16, name=f"wu_{ci}_{im}", tag="wut")
                    nc.gpsimd.dma_start(wut[:], wu_r[:, :, sl])

                    psg = [
                        pp.tile([128, w], f32, name=f"psg{n}_{ci}_{im}", tag=f"psg{n}")
                        for n, (o, w) in enumerate(nsl)
                    ]
                    for kt in range(NT):
                        for n, (o, w) in enumerate(nsl):
                            nc.tensor.matmul(
                                psg[n][:],
                                wgt[:, kt, :],
                                xc[:, kt, o : o + w],
                                start=(kt == 0),
                                stop=(kt == NT - 1),
                            )
                    sg = [
                        sp.tile([128, w], f32, name=f"sg{n}_{ci}_{im}", tag=f"sg{n}")
                        for n, (o, w) in enumerate(nsl)
                    ]
                    for n, (o, w) in enumerate(nsl):
                        nc.scalar.activation(
                            sg[n][:], psg[n][:], mybir.ActivationFunctionType.Silu
                        )
                    psu = [
                        pp.tile([128, w], f32, name=f"psu{n}_{ci}_{im}", tag=f"psu{n}")
                        for n, (o, w) in enumerate(nsl)
                    ]
                    for kt in range(NT):
                        for n, (o, w) in enumerate(nsl):
                            nc.tensor.matmul(
                                psu[n][:],
                                wut[:, kt, :],
                                xc[:, kt, o : o + w],
                                start=(kt == 0),
                                stop=(kt == NT - 1),
                            )
                    for n, (o, w) in enumerate(nsl):
                        nc.vector.tensor_mul(
                            hc[:, im, o : o + w], sg[n][:], psu[n][:]
                        )

                # ---- phase B: y = h @ wd ----
                for hm in range(NT):
                    sl = slice(hm * 128, (hm + 1) * 128)
                    wdt = wp.tile([128, NT, 128], bf16, name=f"wd_{ci}_{hm}", tag="wdt")
                    nc.gpsimd.dma_start(wdt[:], wd_r[:, :, sl])
                    psy = [
                        pp.tile([128, w], f32, name=f"psy{n}_{ci}_{hm}", tag=f"psg{n}")
                        for n, (o, w) in enumerate(nsl)
                    ]
                    for it in range(NT):
                        for n, (o, w) in enumerate(nsl):
                            nc.tensor.matmul(
                                psy[n][:],
                                wdt[:, it, :],
                                hc[:, it, o : o + w],
                                start=(it == 0),
                                stop=(it == NT - 1),
                            )
                    yt = sp.tile([128, CH], f32, name=f"yt_{ci}_{hm}", tag="yt")
                    for n, (o, w) in enumerate(nsl):
                        nc.scalar.copy(yt[:, o : o + w], psy[n][:])
                    nc.sync.dma_start(yT[sl, t0 : t0 + CH], yt[:])
                t0 += CH

    nc.compile()
    return nc


def _get_nc(C, chunks):
    key = (C, chunks)
    if key not in _kernel_cache:
        _kernel_cache[key] = _build(C, chunks)
    return _kernel_cache[key]


def kernel(x, router_w, router_b, w_gate, w_up, w_down):
    global last_results
    from concourse.bass_utils import run_bass_kernel_spmd

    x = np.asarray(x, dtype=np.float32)
    router_w = np.asarray(router_w, dtype=np.float32)
    router_b = np.asarray(router_b, dtype=np.float32)
    w_gate = np.asarray(w_gate, dtype=np.float32)
    w_up = np.asarray(w_up, dtype=np.float32)
    w_down = np.asarray(w_down, dtype=np.float32)

    B, S, _ = x.shape
    T = B * S
    xf = x.reshape(T, H)

    # ---- router on host (0.01% of FLOPs) ----
    logits = xf @ router_w.T + router_b  # (T, E) f32
    idx4 = np.argpartition(logits, E - TOP_K, axis=1)[:, E - TOP_K :]
    topv = np.take_along_axis(logits, idx4, 1)
    mx = topv.max(1, keepdims=True)
    ex = np.exp(topv - mx)
    aff4 = (ex / ex.sum(1, keepdims=True)).astype(np.float32)
    aff = np.zeros((T, E), np.float32)
    np.put_along_axis(aff, idx4, aff4, 1)

    idx = [np.nonzero(aff[:, e])[0] for e in range(E)]
    counts = [len(i) for i in idx]
    C = max(MIN_CAP, -(-max(counts) // 128) * 128)
    ch0 = -(-(C // 2) // 128) * 128
    chunks = (ch0, C - ch0)

    nc = _get_nc(C, chunks)

    # ---- build per-core inputs ----
    xfT = np.ascontiguousarray(xf.T)  # (H, T)
    in_maps = []
    for e in range(E):
        n_e = counts[e]
        xT_e = np.zeros((D, C), np.float32)
        xT_e[:H, :n_e] = xfT[:, idx[e]]
        wgp = np.zeros((D, D), np.float32)
        wgp[:H, :H] = w_gate[e]
        wup = np.zeros((D, D), np.float32)
        wup[:H, :H] = w_up[e]
        wdp = np.zeros((D, D), np.float32)
        wdp[:H, :H] = w_down[e]
        in_maps.append({"xT": xT_e, "wg": wgp, "wu": wup, "wd": wdp})

    res = run_bass_kernel_spmd(nc, in_maps, list(range(N_CORES)))
    last_results = res

    # ---- combine ----
    out = np.zeros((T, H), np.float32)
    for e in range(E):
        n_e = counts[e]
        y = res.results[e]["yT"]  # (D, C) f32
        out[idx[e]] += aff[idx[e], e][:, None] * y[:H, :n_e].T
    return out.reshape(B, S, H)
